# revision 1
# baseline (speedup 1.0000x reference)
"""LocallyConnected1d Bass kernel for 8 TRN2 NeuronCores.

Problem: x [64, 64, 512] f32, weight [1, 64, 64, 504, 9] f32
         out[b, o, l] = sum_{i,k} x[b, i, l+k] * weight[0, o, i, l, k]

Strategy:
  - Shard L_out=504 across 8 cores (63 positions each); x gets a 71-col halo.
  - Per position l the contraction is (i, k) = 576 wide with its own weight
    matrix. Split k into 4 pairs + 1 single (zero-padded to a uniform 5 slots
    of 128 contraction rows).
  - Stationary operand = x-column pair tile [128, 64]: rows 0-63 = x[:, :, p]
    transposed to (i, b), rows 64-127 = x[:, :, p+1]. The moving operand is
    the per-position weight slot [128, 64 (c_out)]. Each position accumulates
    5 matmuls in PSUM (out[b, o]).
  - Even/odd positions write PSUM partitions 0-63 / 64-127 -> PE column
    groups 0/1 run concurrently.
  - Inputs are pre-transposed/padded on host so every DMA is contiguous.
"""

import numpy as np
import ml_dtypes

B = 64
CI = 64
CO = 64
K = 9
L = 512
L_OUT = 504
N_CORES = 8
LP = L_OUT // N_CORES          # 63 positions per core
HALO = LP + K - 1              # 71 x-columns per core
NSLOT = 5                      # 4 k-pairs + 1 single (zero-padded)
NGRP = (LP + 1) // 2           # 32 psum groups of 2 positions
# weight DMA chunks: exactly 8 HWDGE DMAs total (x2 + 5 chunks + 2 out)
# so the 8 DMA semaphore lanes are never reused (a reused lane adds a
# second wait to the out-DMA, exceeding the 1-wait ISA limit).
CHUNK_STARTS = [0, 21, 42]
CHUNK_ENDS = [21, 42, 63]
NWCHUNK = len(CHUNK_STARTS)
CHUNK_OF = [0] * LP
for _c, (_s, _e) in enumerate(zip(CHUNK_STARTS, CHUNK_ENDS)):
    for _l in range(_s, _e):
        CHUNK_OF[_l] = _c

USE_BF16 = True


def _build_bass():
    import concourse.bass as bass
    import concourse.mybir as mybir
    from concourse.tile import TileContext

    dt = mybir.dt.bfloat16 if USE_BF16 else mybir.dt.float32
    nc = bass.Bass()

    x2_d = nc.dram_tensor("x2", [128, HALO * B], dt, kind="ExternalInput")
    wp_d = nc.dram_tensor("wp", [128, LP * NSLOT * CO], dt, kind="ExternalInput")
    out_d = nc.dram_tensor("out", [128, NGRP * CO], mybir.dt.float32,
                           kind="ExternalOutput")

    with TileContext(nc) as tc:
        with (
            tc.tile_pool(name="xc", bufs=1) as xpool,
            tc.tile_pool(name="wc", bufs=NWCHUNK) as wpool,
            tc.tile_pool(name="ps", bufs=1, space="PSUM") as ppool,
            tc.tile_pool(name="ob", bufs=1) as opool,
        ):
            x2 = xpool.tile([128, HALO * B], dt)
            nc.sync.dma_start(out=x2, in_=x2_d[:, :])

            wtiles = []
            for c in range(NWCHUNK):
                lo = CHUNK_STARTS[c] * NSLOT * CO
                hi = CHUNK_ENDS[c] * NSLOT * CO
                t = wpool.tile([128, hi - lo], dt, name=f"wt{c}", tag="wt")
                nc.sync.dma_start(out=t, in_=wp_d[:, lo:hi])
                wtiles.append(t)

            out_sb = opool.tile([128, NGRP * CO], mybir.dt.float32)
            # l=63 does not exist: zero the never-written odd half of the
            # last column group so the out-DMA reads initialized memory.
            nc.vector.memset(
                out_sb[64:128, (NGRP - 1) * CO:NGRP * CO], 0.0)

            # 8 static single-bank PSUM tiles. Position l uses bank l%8,
            # partition half l%2 (-> PE column group l%2, so consecutive
            # position bursts overlap in the array). A position's 5 matmuls
            # run back-to-back: only one accumulation group per PSUM bank
            # "zero region" is ever open (HW constraint), and concurrent
            # copy/matmul never touch the same bank (P10 hazard).
            ptile = [ppool.tile([128, CO], mybir.dt.float32, name=f"pb{t}")
                     for t in range(8)]

            for l in range(LP):
                c = CHUNK_OF[l]
                if l == CHUNK_STARTS[c] and c > 0:
                    # dummy ldweights absorbs the weight-chunk DMA wait so
                    # the following matmul keeps <=1 wait (ISA limit).
                    nc.tensor.ldweights(weights=wtiles[c][:, 0:CO])
                half = l % 2
                outp = ptile[l % 8][half * 64:half * 64 + 64, :]
                for s in range(NSLOT):
                    q = l + 2 * s                 # stationary x column
                    lhsT = x2[:, q * B:(q + 1) * B]
                    off = ((l - CHUNK_STARTS[c]) * NSLOT + s) * CO
                    rhs = wtiles[c][:, off:off + CO]
                    nc.tensor.matmul(outp, lhsT, rhs,
                                     start=(s == 0), stop=(s == NSLOT - 1))
                nc.vector.tensor_copy(
                    out=out_sb[half * 64:half * 64 + 64,
                               (l // 2) * CO:(l // 2 + 1) * CO],
                    in_=outp)
                if l == 31:
                    nc.sync.dma_start(
                        out=out_d[:, :NGRP // 2 * CO],
                        in_=out_sb[:, :NGRP // 2 * CO])
                elif l == LP - 1:
                    nc.sync.dma_start(
                        out=out_d[:, NGRP // 2 * CO:],
                        in_=out_sb[:, NGRP // 2 * CO:])
    _split_multi_waits(nc, mybir)
    return nc


def _split_multi_waits(nc, mybir):
    """This walrus build encodes at most ONE sync wait per instruction.

    Tile attaches multi-wait lists (e.g. on the kernel-tail Drain). Hoist
    all but the last wait onto single-wait NoOps inserted just before the
    instruction on the same engine -- semantically identical (the engine
    stalls at the NoOps instead of at the instruction itself).
    """
    for f in nc.m.functions:
        for bb in f.blocks:
            out = []
            for inst in bb.instructions:
                si = inst.sync_info
                waits = list(si.on_wait) if si is not None and si.on_wait else []
                if len(waits) > 1:
                    for k, w in enumerate(waits[:-1]):
                        out.append(mybir.InstNoOp(
                            name=f"{inst.name}-wsplit{k}",
                            engine=inst.engine,
                            sync_info=mybir.SyncInfo(on_wait=[w], on_update=[]),
                            bass_nofuse=True))
                    inst.sync_info = mybir.SyncInfo(
                        on_wait=[waits[-1]],
                        on_update=list(si.on_update) if si.on_update else [])
                out.append(inst)
            bb.instructions = out


def _prep_inputs(x, weight):
    """Returns list of 8 per-core input dicts."""
    npdt = ml_dtypes.bfloat16 if USE_BF16 else np.float32
    x = np.asarray(x, np.float32)
    w0 = np.asarray(weight, np.float32)[0]        # [CO, CI, L_OUT, K]

    wt = np.ascontiguousarray(w0.transpose(2, 3, 1, 0))   # [L_OUT, K, CI, CO]
    wslots = np.zeros((L_OUT, NSLOT, 128, CO), np.float32)
    wslots[:, :4] = wt[:, :8].reshape(L_OUT, 4, 128, CO)
    wslots[:, 4, :CI] = wt[:, 8]

    xt = x.transpose(1, 2, 0)                     # [CI, L, B]

    in_maps = []
    for m in range(N_CORES):
        hs = LP * m
        x2 = np.zeros((128, HALO, B), np.float32)
        x2[:CI] = xt[:, hs:hs + HALO]
        x2[CI:, :HALO - 1] = xt[:, hs + 1:hs + HALO]
        wp = wslots[hs:hs + LP].transpose(2, 0, 1, 3)     # [128, LP, NSLOT, CO]
        in_maps.append({
            "x2": np.ascontiguousarray(x2.reshape(128, HALO * B)).astype(npdt),
            "wp": np.ascontiguousarray(wp).reshape(128, LP * NSLOT * CO).astype(npdt),
        })
    return in_maps


def _decode_outputs(results):
    outs = []
    for r in results:
        v = np.asarray(r["out"], np.float32).reshape(2, 64, NGRP, CO)
        # v[h, b, g, o] -> out[b, o, l], l = 2g + h
        t = v.transpose(1, 3, 2, 0).reshape(B, CO, NGRP * 2)[:, :, :LP]
        outs.append(t)
    return np.concatenate(outs, axis=2)           # [B, CO, L_OUT]


_CACHED_NC = None


def kernel(x, weight):
    global _CACHED_NC
    from concourse.bass_utils import run_bass_kernel_spmd

    if _CACHED_NC is None:
        _CACHED_NC = _build_bass()
    in_maps = _prep_inputs(x, weight)
    res = run_bass_kernel_spmd(_CACHED_NC, in_maps, core_ids=list(range(N_CORES)))
    return _decode_outputs(res.results)



# revision 2
# speedup vs baseline: 96298.4834x; 96298.4834x over previous
"""LocallyConnected1d Bass kernel for 8 TRN2 NeuronCores (v3).

Problem: x [64, 64, 512] f32, weight [1, 64, 64, 504, 9] f32
         out[b, o, l] = sum_{i,k} x[b, i, l+k] * weight[0, o, i, l, k]

Per core (L_out sharded 8 x 63):
  - Contraction (i, k) = 576 = 4 k-pair matmuls of 128 rows + 1 single
    (k=8) of 64 rows.  No zero padding: the k=8 plane is packed two
    positions per 128 partitions (even l on 0:64, odd l on 64:128; the
    odd-l stationary reads x2's bottom half, col l+7 = x col l+8).
  - x2 is the host-duplicated pair layout [128, 71*64] (col q rows 64:128
    = x col q+1) loaded in 3 column chunks.
  - Position l accumulates 5 matmuls into psum bank l//16, column slot
    (l//2)%8, partition half (l%2)*64; 4 banks hold all 63 outputs, so 4
    big DVE cast-copies (f32 -> bf16) drain PSUM, then bf16 out DMAs.
  - DMAs are spread over the three DMA-capable rings (SP / ACT / Pool)
    so transfers overlap; out DMAs ride ACT after its weight chunks.
"""

import numpy as np
import ml_dtypes

B = 64
CI = 64
CO = 64
K = 9
L = 512
L_OUT = 504
N_CORES = 8
LP = L_OUT // N_CORES          # 63 positions per core
HALO = LP + K - 1              # 71 x-columns per core
NPAIR = 4                      # k-pairs 0..7 -> 4 slots of 128 rows
NGRP = 32                      # column groups total (l//2)
# PSUM banks hold whole column-group ranges; a bank must be fully closed
# (all its positions' matmuls done) before its drain copy, and any write
# to a bank serializes against copies of that bank (P10 hazard), so the
# final bank is tiny to keep the tail short.
BANK_G = [(0, 7), (7, 14), (14, 21), (21, 28), (28, 30), (30, 32)]
NBANK = len(BANK_G)
COPY_AFTER = [13, 27, 41, 55, 59, 62]       # last l of each bank

WCHUNKS = [(0, 3), (3, 9), (9, 16), (16, 24), (24, 33), (33, 43), (43, 53),
           (53, 63)]
W8CHUNKS = [(0, 8), (8, 20), (20, 32)]      # in groups g = l//2
XCHUNKS = [(0, 10), (10, 28), (28, 48), (48, 71)]  # x2 column ranges
NWARM = 40                     # PE p-state warm-up dummy matmuls

OUT_COLS = NGRP * CO           # 2048 bf16 cols

SP, ACT, POOL = "sync", "scalar", "gpsimd"
# ring assignment: (ring, kind, index) in per-ring program order
DMA_PLAN = {
    SP: [("wp", 0), ("w8", 0), ("wp", 2), ("w8", 1), ("wp", 4), ("w8", 2),
         ("wp", 6)],
    ACT: [("wp", 1), ("wp", 3), ("wp", 5), ("wp", 7)],
    POOL: [("x2", 0), ("x2", 1), ("x2", 2), ("x2", 3)],
}


def _build_bass():
    import concourse.bass as bass
    import concourse.mybir as mybir
    from concourse.tile import TileContext

    dt = mybir.dt.bfloat16
    nc = bass.Bass()

    x2_d = nc.dram_tensor("x2", [128, HALO * B], dt, kind="ExternalInput")
    wp_d = nc.dram_tensor("wp", [128, LP * NPAIR * CO], dt, kind="ExternalInput")
    w8_d = nc.dram_tensor("w8", [128, NGRP * CO], dt, kind="ExternalInput")
    out_d = nc.dram_tensor("out", [128, OUT_COLS], dt, kind="ExternalOutput")

    with TileContext(nc) as tc:
        with (
            tc.tile_pool(name="xc", bufs=1) as xpool,
            tc.tile_pool(name="wc", bufs=1) as wpool,
            tc.tile_pool(name="ps", bufs=1, space="PSUM") as ppool,
            tc.tile_pool(name="ob", bufs=1) as opool,
        ):
            x2 = xpool.tile([128, HALO * B], dt)
            wtiles = [wpool.tile([128, (e - s) * NPAIR * CO], dt, name=f"wt{c}")
                      for c, (s, e) in enumerate(WCHUNKS)]
            w8tiles = [wpool.tile([128, (e - s) * CO], dt, name=f"w8t{c}")
                       for c, (s, e) in enumerate(W8CHUNKS)]



            def issue(ring, kind, i):
                eng = getattr(nc, ring)
                if kind == "wp":
                    s, e = WCHUNKS[i]
                    eng.dma_start(out=wtiles[i],
                                  in_=wp_d[:, s * NPAIR * CO:e * NPAIR * CO])
                elif kind == "w8":
                    s, e = W8CHUNKS[i]
                    eng.dma_start(out=w8tiles[i], in_=w8_d[:, s * CO:e * CO])
                elif kind == "x2":
                    s, e = XCHUNKS[i]
                    eng.dma_start(out=x2[:, s * B:e * B],
                                  in_=x2_d[:, s * B:e * B])

            # interleave ring issue so each ring's first DMA is early
            maxlen = max(len(v) for v in DMA_PLAN.values())
            for j in range(maxlen):
                for ring in (POOL, SP, ACT):
                    if j < len(DMA_PLAN[ring]):
                        issue(ring, *DMA_PLAN[ring][j])

            out_sb = opool.tile([128, OUT_COLS], dt)
            # full-bank tiles regardless of use, so no two banks share a
            # physical PSUM bank (copies serialize against same-bank writes)
            psum = [ppool.tile([128, 512], mybir.dt.float32,
                               name=f"pb{t}") for t in range(NBANK)]
            # l=63 does not exist; pre-zero its psum slot (bank 5, group 31,
            # partitions 64:128) so the final copy reads initialized memory.
            nc.vector.memset(psum[5][64:128, 64:128], 0.0)

            def chunk_of(l, chunks):
                for c, (s, e) in enumerate(chunks):
                    if s <= l < e:
                        return c, s
                raise AssertionError

            for l in range(LP):
                g = l // 2
                bank, bs = chunk_of(g, BANK_G)
                slot = g - bs
                half = l % 2
                outp = psum[bank][half * 64:half * 64 + 64,
                                  slot * CO:(slot + 1) * CO]
                c, cs = chunk_of(l, WCHUNKS)
                for s in range(NPAIR):
                    q = l + 2 * s
                    lhsT = x2[:, q * B:(q + 1) * B]
                    off = ((l - cs) * NPAIR + s) * CO
                    rhs = wtiles[c][:, off:off + CO]
                    nc.tensor.matmul(outp, lhsT, rhs,
                                     start=(s == 0), stop=False)
                # k=8 single: even l reads top halves, odd l bottom halves
                c8, cs8 = chunk_of(g, W8CHUNKS)
                po = half * 64
                q8 = l + 8 - half            # bottom col q holds x col q+1
                lhsT8 = x2[po:po + 64, q8 * B:(q8 + 1) * B]
                rhs8 = w8tiles[c8][po:po + 64, (g - cs8) * CO:(g - cs8 + 1) * CO]
                nc.tensor.matmul(outp, lhsT8, rhs8, start=False, stop=True)

                # Drain a fully-closed bank: big cast-copy (f32 -> bf16),
                # then its out-DMA on a ring that's idle by then.  The bank
                # b copy overlaps bank b+1's matmuls (different banks).
                if l == COPY_AFTER[bank]:
                    lo, hi = bs * CO, BANK_G[bank][1] * CO
                    nc.vector.tensor_copy(
                        out=out_sb[:, lo:hi], in_=psum[bank][:, :hi - lo])
                    ring = (nc.gpsimd, nc.gpsimd, nc.gpsimd, nc.gpsimd,
                            nc.sync, nc.scalar)[bank]
                    ring.dma_start(out=out_d[:, lo:hi],
                                   in_=out_sb[:, lo:hi])
    _split_multi_waits(nc, mybir)
    return nc


def _split_multi_waits(nc, mybir):
    """This walrus build encodes at most ONE sync wait per instruction.

    Tile attaches multi-wait lists (e.g. on the kernel-tail Drain). Hoist
    all but the last wait onto single-wait NoOps inserted just before the
    instruction on the same engine -- semantically identical (the engine
    stalls at the NoOps instead of at the instruction itself).
    """
    for f in nc.m.functions:
        for bb in f.blocks:
            out = []
            for inst in bb.instructions:
                si = inst.sync_info
                waits = list(si.on_wait) if si is not None and si.on_wait else []
                if len(waits) > 1:
                    for k, w in enumerate(waits[:-1]):
                        out.append(mybir.InstNoOp(
                            name=f"{inst.name}-wsplit{k}",
                            engine=inst.engine,
                            sync_info=mybir.SyncInfo(on_wait=[w], on_update=[]),
                            bass_nofuse=True))
                    inst.sync_info = mybir.SyncInfo(
                        on_wait=[waits[-1]],
                        on_update=list(si.on_update) if si.on_update else [])
                out.append(inst)
            bb.instructions = out


def _prep_inputs(x, weight):
    """Returns list of 8 per-core input dicts."""
    npdt = ml_dtypes.bfloat16
    x = np.asarray(x, np.float32)
    w0 = np.asarray(weight, np.float32)[0]        # [CO, CI, L_OUT, K]

    wt = np.ascontiguousarray(w0.transpose(2, 3, 1, 0))   # [L_OUT, K, CI, CO]
    # [L_OUT, 4, 128, CO]: row p*64+i of slot s = w[o, i, l, k=2s+p]
    wpf = wt[:, :8].reshape(L_OUT, NPAIR, 128, CO)
    w8f = wt[:, 8]                                # [L_OUT, CI, CO]

    xt = np.ascontiguousarray(x.transpose(1, 2, 0)).astype(npdt)  # [CI, L, B]

    in_maps = []
    for m in range(N_CORES):
        hs = LP * m
        x2 = np.zeros((128, HALO, B), npdt)
        x2[:CI] = xt[:, hs:hs + HALO]
        x2[CI:, :HALO - 1] = xt[:, hs + 1:hs + HALO]
        wp = wpf[hs:hs + LP].transpose(2, 0, 1, 3)    # [128, LP, 4, CO]
        w8 = np.zeros((2, CI, NGRP, CO), np.float32)
        w8[0] = w8f[hs:hs + LP:2].transpose(1, 0, 2)          # even l
        w8[1, :, :LP // 2] = w8f[hs + 1:hs + LP:2].transpose(1, 0, 2)
        in_maps.append({
            "x2": np.ascontiguousarray(x2.reshape(128, HALO * B)),
            "wp": np.ascontiguousarray(wp).reshape(128, LP * NPAIR * CO)
                    .astype(npdt),
            "w8": np.ascontiguousarray(w8.reshape(128, NGRP * CO)).astype(npdt),
        })
    return in_maps


def _decode_outputs(results):
    outs = []
    for r in results:
        v = np.asarray(r["out"]).astype(np.float32)
        # [h*64+b, g*64+o] -> out[b, o, l], l = 2g+h
        t = (v.reshape(2, 64, NGRP, CO)
             .transpose(1, 3, 2, 0)
             .reshape(B, CO, NGRP * 2)[:, :, :LP])
        outs.append(t)
    return np.concatenate(outs, axis=2).astype(np.float32)  # [B, CO, L_OUT]


_CACHED_NC = None


def kernel(x, weight):
    global _CACHED_NC
    from concourse.bass_utils import run_bass_kernel_spmd

    if _CACHED_NC is None:
        _CACHED_NC = _build_bass()
    in_maps = _prep_inputs(x, weight)
    res = run_bass_kernel_spmd(_CACHED_NC, in_maps, core_ids=list(range(N_CORES)))
    return _decode_outputs(res.results)
